# revision 33
# baseline (speedup 1.0000x reference)
"""BasesDecomposition (R-GCN style) message passing kernel for Trainium2.

V4 strategy (8 NeuronCores, SPMD — one program, per-core data):
  - Nodes sharded by row: core c owns targets [c*NL, (c+1)*NL).
  - Edges symmetrized on host, partitioned by target-owner core, then by
    pipeline third (target-block range) and relation.
  - Host pre-gathers source features: XST[:, slot] = ew_e * x[src_e]
    (transposed, fp16, edge-weight folded in) so the device does no
    per-edge gathers and no transposes in phase 1.
  - Phase 1 (messages): per 128-edge relation-pure chunk, one fp16
    matmul XST_chunk.T @ W_r -> PSUM (4 chunks per PSUM bank, one
    fp32->fp16 copy each, alternating vector/scalar engines). Up to WG
    chunks share ONE grouped DMA write to the message buffer md. The md
    slot map row = rowbase_g + e*n_g + j keeps the write contiguous per
    partition while ranks (target-block-sorted within each relation)
    stay contiguous for phase-2 interval gathers.
  - Phase 2 (aggregate): per 128-target block, ONE indirect interval
    gather (128 intervals x SL rows) covering the block's per-relation
    runs; ONE broadcast is_equal builds all SL one-hot matrices; SL
    scatter matmuls accumulate out[t, o] += T_j.T @ M_j in PSUM, plus a
    self-loop matmul; fp16 copy + direct DMA to the output.
  - H pipeline stages: phase 2 of stage h-1 is interleaved with phase 1
    of stage h so DMA/gpsimd/PE/DVE work overlaps across stages.
"""

import numpy as np
import ml_dtypes

import concourse.bass as bass
import concourse.bacc as bacc
import concourse.tile as tile
import concourse.mybir as mybir
from concourse.bass_utils import run_bass_kernel_spmd

F8 = mybir.dt.float8e4
F16 = mybir.dt.float16
F32 = mybir.dt.float32
I32 = mybir.dt.int32
F8NP = ml_dtypes.float8_e4m3fn

NCORE = 8
H = 3            # pipeline stages (target-block ranges)
HW_ = (0.3, 0.4, 0.3)  # stage size fractions (taper head/tail)
WG = 16          # chunks per md write group (slot-map group size)
PG = 4           # chunks per PSUM bank / cast
KR = 16          # chunks per XST read DMA
OB = 8           # output blocks per batched write
SLC = (6, 7, 8, 10, 12, 16)  # per-block md rows per cover interval


def _ranks_within_group(keys, order, nbins):
    counts = np.bincount(keys, minlength=nbins)
    starts = np.concatenate([[0], np.cumsum(counts)[:-1]])
    r = np.empty(len(keys), np.int64)
    r[order] = np.arange(len(keys)) - starts[keys[order]]
    return r


def host_prep(x, node_keep_mask, source, target, edge_type, edge_weights,
              bases, relation_base_weights):
    n, d = x.shape
    assert d == 128 and n % NCORE == 0
    R = relation_base_weights.shape[0] - 1
    nl = n // NCORE
    nblk = (nl + 127) // 128
    nlp = nblk * 128
    # H block ranges, tapered so first/last stages are smaller
    cw = np.cumsum((0.0,) + HW_) / sum(HW_)
    bnds = [round(nblk * float(c)) for c in cw]
    hb = [(bnds[i], bnds[i + 1]) for i in range(H)]
    f16, f32 = np.float16, np.float32

    W = np.einsum("rb,bdo->rdo", relation_base_weights.astype(f32),
                  bases.astype(f32)).astype(f32)
    wsb_h = np.ascontiguousarray(
        W.transpose(1, 0, 2).reshape(d, (R + 1) * d)).astype(f16)

    x16 = x.astype(f16).astype(f32)  # quantize once, scale in f32
    src2 = np.concatenate([source, target]).astype(np.int64)
    tgt2 = np.concatenate([target, source]).astype(np.int64)
    et2 = np.concatenate([edge_type, edge_type]).astype(np.int64)
    ew2 = np.concatenate([edge_weights, edge_weights]).astype(f32)

    owner = tgt2 // nl
    tloc = tgt2 - owner * nl
    blkg = tloc // 128
    tin = (tloc - blkg * 128).astype(f32)
    halfid = np.digitize(blkg, bnds[1:-1])

    cnt_chr = np.bincount(
        (owner * H + halfid) * R + et2, minlength=NCORE * H * R
    ).reshape(NCORE, H, R)

    cnt_chrb = np.bincount(
        ((owner * H + halfid) * R + et2) * nblk + blkg,
        minlength=NCORE * H * R * nblk).reshape(NCORE, H, R, nblk)

    halves = []
    for h in range(H):
        b0, b1 = hb[h]
        # per-block minimal interval stride (shared across cores)
        SLs = []
        for b in range(b0, b1):
            for SL in SLC:
                n_iv = int(np.ceil(cnt_chrb[:, h, :, b] / SL)
                           .sum(axis=1).max())
                if n_iv <= 128:
                    break
            else:
                raise AssertionError(f"no SL fits: {n_iv}")
            SLs.append(int(SL))
        SLmax_h = max(SLs)
        toffs = [0]
        for s in SLs:
            toffs.append(toffs[-1] + s)
        nch_r = np.ceil((cnt_chr[:, h].max(axis=0) + SLmax_h) / 128.0
                        ).astype(np.int64)
        cb = np.concatenate([[0], np.cumsum(nch_r)]).astype(np.int64)
        nch_h = int(cb[-1])
        groups = []
        for r in range(R):
            for g0 in range(0, int(nch_r[r]), WG):
                ng_ = int(min(WG, int(nch_r[r]) - g0))
                cf = int(cb[r] + g0)
                groups.append((128 * cf, cf, ng_, r))
        halves.append(dict(nch=nch_h, ep1=128 * nch_h, cb=cb, nch_r=nch_r,
                           groups=tuple(groups), b0=b0, b1=b1,
                           nbh=b1 - b0, SLs=tuple(SLs),
                           toffs=tuple(toffs)))

    per_core = []
    for c in range(NCORE):
        dcore = {"wsb": wsb_h}
        xm = (x16[c * nl:(c + 1) * nl]
              * node_keep_mask[c * nl:(c + 1) * nl, None])
        xmt = np.zeros((128, nlp), f16)
        xmt[:, :nl] = xm.T.astype(f16)
        dcore["xmt"] = np.ascontiguousarray(xmt)
        for h in range(H):
            hs = halves[h]
            b0, nbh = hs["b0"], hs["nbh"]
            SLs, toffs = hs["SLs"], hs["toffs"]
            cbs, nch_r, nch_h, ep1 = (hs["cb"], hs["nch_r"], hs["nch"],
                                      hs["ep1"])
            sel = np.nonzero((owner == c) & (halfid == h))[0]
            et_s = et2[sel]
            blk_s = blkg[sel] - b0
            order = np.lexsort((blk_s, et_s))
            ranks = _ranks_within_group(et_s, order, R)
            rows = 128 * cbs[et_s] + ranks
            nch_of = nch_r[et_s]
            g = ranks // (128 * WG)
            ng_of = np.minimum(WG, nch_of - WG * g)
            om = ranks - g * 128 * WG
            e = om // ng_of
            jj = om - e * ng_of
            chunk = cbs[et_s] + WG * g + jj
            xcol = chunk * 128 + e
            XS = np.zeros((128 * nch_h, d), f16)
            XS[xcol] = (x16[src2[sel]] * ew2[sel][:, None]).astype(f16)
            dcore[f"xst{h}"] = np.ascontiguousarray(XS.T)

            edge_of_row = np.full(ep1, -1, np.int64)
            edge_of_row[rows] = sel
            cnt_rb = cnt_chrb[c, h, :, b0:hs["b1"]]
            run_start = np.zeros_like(cnt_rb)
            run_start[:, 1:] = np.cumsum(cnt_rb, axis=1)[:, :-1]
            cidx = np.zeros((128, nbh), np.int32)
            tcol = np.full((128, toffs[-1]), -1.0, f32)
            nuse = 0
            for b in range(nbh):
                SL = SLs[b]
                # pad entries point at tail pad rows (written, no edges)
                cidx[:, b] = ep1 - SL
                iv = []
                for r in range(R):
                    s = 128 * int(cbs[r]) + int(run_start[r, b])
                    ln = int(cnt_rb[r, b])
                    limit = 128 * int(cbs[r] + nch_r[r]) - SL
                    for off in range(0, ln, SL):
                        iv.append(min(s + off, limit))
                assert len(iv) <= 128, f"cover overflow {len(iv)}"
                cidx[:len(iv), b] = iv
                rowsm = cidx[:, b].astype(np.int64)[:, None] + np.arange(SL)
                evm = edge_of_row[rowsm]
                valid = evm >= 0
                evc = np.where(valid, evm, 0)
                use = (valid & (blkg[evc] - b0 == b) & (halfid[evc] == h))
                nuse += int(use.sum())
                tcol[:, toffs[b]:toffs[b + 1]] = np.where(
                    use, tin[evc], -1.0)
            assert nuse == len(sel), f"cover mismatch {nuse} vs {len(sel)}"
            dcore[f"cidx{h}"] = np.ascontiguousarray(cidx)
            dcore[f"tcol{h}"] = np.ascontiguousarray(tcol.astype(f16))
        per_core.append(dcore)

    cfg = dict(R=R, nlp=nlp, nblk=nblk,
               halves=tuple((hs["nch"], hs["ep1"], hs["b0"], hs["b1"],
                             hs["SLs"], hs["toffs"], hs["groups"])
                            for hs in halves))
    return per_core, cfg


def build_program(cfg):
    R = cfg["R"]
    nlp = cfg["nlp"]
    SLmax = max(max(hv[4]) for hv in cfg["halves"])

    nc = bacc.Bacc(None, target_bir_lowering=False, debug=False)
    wsb = nc.declare_dram_parameter("wsb", [128, (R + 1) * 128], F16,
                                    isOutput=False)
    xmt = nc.declare_dram_parameter("xmt", [128, nlp], F16, isOutput=False)
    hp = []
    for h, (nch_h, ep1, b0, b1, SLs, toffs, groups) in enumerate(
            cfg["halves"]):
        xst = nc.declare_dram_parameter(f"xst{h}", [128, nch_h * 128], F16,
                                        isOutput=False)
        cidx = nc.declare_dram_parameter(f"cidx{h}", [128, b1 - b0], I32,
                                         isOutput=False)
        tcol = nc.declare_dram_parameter(f"tcol{h}", [128, toffs[-1]], F16,
                                         isOutput=False)
        md = nc.dram_tensor(f"md{h}", [ep1, 128], F16)
        hp.append((xst, cidx, tcol, md))
    outp = nc.declare_dram_parameter("out", [nlp, 128], F16, isOutput=True)

    colidx_d = nc.inline_tensor(
        np.tile(np.arange(128, dtype=np.float16), (128, SLmax)),
        name="colidx_c")

    with tile.TileContext(nc) as tc:
        with (
            tc.tile_pool(name="const", bufs=1) as constp,
            tc.tile_pool(name="rd", bufs=8) as rdp,
            tc.tile_pool(name="msb", bufs=4) as msbp,
            tc.tile_pool(name="p1ps", bufs=4, space="PSUM") as p1ps,
            tc.tile_pool(name="mg", bufs=12) as mgp,
            tc.tile_pool(name="tt", bufs=8) as ttp,
            tc.tile_pool(name="ob", bufs=4) as obp,
            tc.tile_pool(name="p2ps", bufs=4, space="PSUM") as p2ps,
        ):
            wsb_t = constp.tile([128, (R + 1) * 128], F16)
            nc.sync.dma_start(out=wsb_t[:], in_=wsb[:])
            xmt_t = constp.tile([128, nlp], F16)
            nc.sync.dma_start(out=xmt_t[:], in_=xmt[:])
            colidx = constp.tile([128, SLmax, 128], F16)
            nc.sync.dma_start(out=colidx[:], in_=colidx_d[:])
            cidx_ts, tcol_ts = [], []
            for h, (nch_h, ep1, b0, b1, SLs, toffs, groups) in enumerate(
                    cfg["halves"]):
                nbh = b1 - b0
                ct = constp.tile([128, nbh], I32, name=f"cidx_t{h}")
                nc.sync.dma_start(out=ct[:], in_=hp[h][1][:])
                cidx_ts.append(ct)
                tc_ = constp.tile([128, toffs[-1]], F16, name=f"tcol_t{h}")
                nc.sync.dma_start(out=tc_[:], in_=hp[h][2][:])
                tcol_ts.append(tc_)

            read_cache = [dict() for _ in range(H)]
            alt = [0]  # cast engine alternator
            alt_w = [0]  # md write engine alternator

            def _issue_read(h, bi):
                nch_h = cfg["halves"][h][0]
                nbat = (nch_h + KR - 1) // KR
                rc = read_cache[h]
                if bi >= nbat or bi in rc:
                    return
                w = min(KR, nch_h - bi * KR)
                rt = rdp.tile([128, KR * 128], F16, tag="rt")
                nc.sync.dma_start(
                    out=rt[:, :w * 128],
                    in_=hp[h][0][:, bi * KR * 128:(bi * KR + w) * 128])
                rc[bi] = rt

            def get_read(h, ci):
                bi = ci // KR
                rc = read_cache[h]
                for d in (0, 1, 2):  # prefetch two batches ahead
                    _issue_read(h, bi + d)
                for old in [k for k in rc if k < bi]:
                    del rc[old]
                return rc[bi]

            def emit_p1_group(h, gi):
                groups = cfg["halves"][h][6]
                md_d = hp[h][3]
                rowbase, cf, ng_, rel = groups[gi]
                msb = msbp.tile([128, WG * 128], F16, tag="msb")
                for s0 in range(0, ng_, PG):
                    sn = min(PG, ng_ - s0)
                    mp = p1ps.tile([128, PG * 128], F32, tag="mp")
                    for j in range(sn):
                        ci = cf + s0 + j
                        rt = get_read(h, ci)
                        off = (ci % KR) * 128
                        nc.tensor.matmul(
                            out=mp[:, j * 128:(j + 1) * 128],
                            lhsT=rt[:, off:off + 128],
                            rhs=wsb_t[:, rel * 128:(rel + 1) * 128],
                            start=True, stop=True)
                    if alt[0] % 2 == 0:
                        nc.vector.tensor_copy(
                            out=msb[:, s0 * 128:(s0 + sn) * 128],
                            in_=mp[:, :sn * 128])
                    else:
                        nc.scalar.copy(
                            out=msb[:, s0 * 128:(s0 + sn) * 128],
                            in_=mp[:, :sn * 128])
                    alt[0] += 1
                dst = md_d[rowbase:rowbase + 128 * ng_, :].rearrange(
                    "(e j) o -> e j o", j=ng_)
                weng = nc.scalar if alt_w[0] % 2 == 0 else nc.sync
                alt_w[0] += 1
                weng.dma_start(out=dst, in_=msb[:, :ng_ * 128])

            ob_state = [None, 0]

            def emit_p2_block(h, b):
                nch_h, ep1, b0, b1, SLs, toffs, groups = cfg["halves"][h]
                SL = SLs[b]
                md_d = hp[h][3]
                mg = mgp.tile([128, SLmax * 128], F16, tag="mg")
                nc.gpsimd.indirect_dma_start(
                    out=mg[:, :SL * 128], out_offset=None, in_=md_d[:, :],
                    in_offset=bass.IndirectOffsetOnAxis(
                        ap=cidx_ts[h][:, b:b + 1], axis=0))
                tt = ttp.tile([128, SLmax, 128], F16, tag="tt")
                nc.vector.tensor_tensor(
                    out=tt[:, :SL, :], in0=colidx[:, :SL, :],
                    in1=tcol_ts[h][:, toffs[b]:toffs[b + 1]].unsqueeze(2)
                    .to_broadcast([128, SL, 128]),
                    op=mybir.AluOpType.is_equal)
                ps = p2ps.tile([128, 128], F32, tag="acc")
                for j in range(SL):
                    nc.tensor.matmul(
                        out=ps[:], lhsT=tt[:, j, :],
                        rhs=mg[:, j * 128:(j + 1) * 128],
                        start=(j == 0), stop=False)
                gb = b0 + b
                nc.tensor.matmul(
                    out=ps[:], lhsT=xmt_t[:, gb * 128:(gb + 1) * 128],
                    rhs=wsb_t[:, R * 128:(R + 1) * 128],
                    start=False, stop=True)
                if ob_state[0] is None:
                    ob_state[0] = obp.tile([128, OB * 128], F16, tag="ob",
                                           name="obbig")
                    ob_state[1] = gb
                i = gb - ob_state[1]
                nc.scalar.copy(out=ob_state[0][:, i * 128:(i + 1) * 128],
                               in_=ps[:])
                if i == OB - 1 or b == b1 - b0 - 1:
                    nb = i + 1
                    gb0 = ob_state[1]
                    dst = outp[gb0 * 128:(gb0 + nb) * 128, :].rearrange(
                        "(b t) o -> t b o", b=nb)
                    nc.sync.dma_start(out=dst, in_=ob_state[0][:, :nb * 128])
                    ob_state[0] = None

            # schedule: p1(0); for h>=1: p2(h-1) interleaved with p1(h);
            # then p2(H-1)
            ngs = [len(cfg["halves"][h][6]) for h in range(H)]
            nbs = [cfg["halves"][h][3] - cfg["halves"][h][2]
                   for h in range(H)]
            for gi in range(ngs[0]):
                emit_p1_group(0, gi)
            for h in range(1, H):
                k = 0
                for b in range(nbs[h - 1]):
                    emit_p2_block(h - 1, b)
                    take = ((b + 1) * ngs[h]) // nbs[h - 1] \
                        - (b * ngs[h]) // nbs[h - 1]
                    for _ in range(take):
                        emit_p1_group(h, k)
                        k += 1
                while k < ngs[h]:
                    emit_p1_group(h, k)
                    k += 1
            for b in range(nbs[H - 1]):
                emit_p2_block(H - 1, b)

    nc.finalize()
    return nc


_PROGRAM_CACHE = {}


def _get_program(cfg):
    key = (cfg["R"], cfg["nlp"], cfg["nblk"], cfg["halves"])
    if key not in _PROGRAM_CACHE:
        _PROGRAM_CACHE[key] = build_program(cfg)
    return _PROGRAM_CACHE[key]


def kernel(x, node_keep_mask, source, target, edge_type, edge_weights,
           bases, relation_base_weights):
    per_core, cfg = host_prep(x, node_keep_mask, source, target, edge_type,
                              edge_weights, bases, relation_base_weights)
    nc = _get_program(cfg)
    res = run_bass_kernel_spmd(nc, per_core, list(range(NCORE)))
    n = x.shape[0]
    nl = n // NCORE
    out = np.empty((n, 128), np.float32)
    for c in range(NCORE):
        out[c * nl:(c + 1) * nl] = res.results[c]["out"][:nl].astype(
            np.float32)
    return out
